# revision 22
# baseline (speedup 1.0000x reference)
"""Trainium2 Bass kernel for nn_BSLoss (Black-Scholes PINN loss on a 4096x4096 grid).

Strategy V3 (8 NeuronCores, SPMD, S-sharded, fp16 on device):
  - Host converts V to fp16 (halves DMA: ~4.4 MB/core). The loss is a mean of
    ~16.7M squared residuals, so V-rounding noise enters as E[eps^2]/E[r^2]
    ~ 2^-22 -- far below the 2e-2 gate.
  - Each core: 512 grid rows (+1-row halos) x 4096 t-cols as 4 x [128, 4096]
    tiles (output rows 1..126 each) + the last 10 rows folded as 2
    column-chunks x 10 rows = [20, 2049] (4094 = 2*2047 exactly, so the fold
    has no garbage columns).
  - Work unit = supergroup of 2048 output cols ([128, 2048] PSUM, 4 banks,
    ring of 2). Per supergroup: DVE computes D = V[:,t+1]-V[:,t-1] in one
    wide fp16 TT (2x mode); PE runs 4x512 tridiagonal fp16 matmuls
    (S-stencil) then 4 identity matmuls accumulating D into the same banks;
    the consumer is either one wide ScalarE activation(Square, accum_out) or
    4 DVE bn_stats (chosen per supergroup to balance ACT vs DVE).
  - DMA issue is spread across the SP, Activation, and GPSIMD DGE queues so
    descriptor generation doesn't serialize behind one queue.
  - Host applies per-row masks (x C_T^2, the folded-out time-step scale),
    reconstructs bn sums (M2 + n*mean^2), and computes the O(N) boundary
    losses in float64.
"""
import os
import sys

if "/opt/trn_rl_repo" not in sys.path:
    sys.path.insert(0, "/opt/trn_rl_repo")

import numpy as np

import concourse.mybir as mybir
import concourse.tile as tile
from concourse import bacc
from concourse.bass_utils import run_bass_kernel_spmd

# ---- problem constants (match the reference) ----
N_S, N_T = 4096, 4096
R, SIGMA, K, T_MAT, SMAX = 0.05, 0.2, 100.0, 1.0, 300.0
B_STR, ALPHA = K / SMAX, 0.5
L_PDE, L_BC, L_TC = 1.0, 10.0, 10.0
HUBER_DELTA = 0.01
SOFTPLUS_BETA = 50.0

N_CORES = 8
ROWS_PER_CORE = N_S // N_CORES          # 512
IN_ROWS = ROWS_PER_CORE + 2             # 514 (with halos)
P = 128
TILE_STARTS = [0, 126, 252, 378]        # full tiles; outputs local rows 1..504
STRIP_START = 504                       # strip rows 504..513 -> outputs 505..512
C_T = (N_T - 1) / 2.0 / T_MAT           # 2047.5

# weight layout (fp16): 4 tri blocks, identity, folded strip tri
W_IDENT = 512                           # cols 512..640: eye(128)
W_STRIP = 640                           # cols 640..660: [20,20] block-tridiag
W_COLS = 768

# main groups: output cols c0..c0+w-1, 1024-wide (1 PSUM bank pair, ring 4)
GROUP_C0 = [1, 1025, 2049, 3073]
GROUP_W = [1024, 1024, 1024, 1022]
# t-diff subs are emitted per pair (2048 wide); DMA halves overlap by 2 cols
# so the front pair's sub needs only half 0
PAIR_C0 = [1, 2049]
PAIR_W = [2048, 2046]
H0_W = 2050
# strip folded as 2 column-chunks x 10 rows: outputs f=1..2047 per chunk,
# chunk j <-> global cols 2047*j + f
STRIP_W = 2047
# groups whose sum-of-squares runs on DVE bn_stats instead of ACT (two in
# tile 3 so the tail isn't squeezed through ACT alone)
BN_GROUPS = [(0, 2), (1, 0), (1, 2), (2, 2), (3, 1), (3, 3)]
# tiles whose t-diff subs run on the (slow but otherwise idle) Pool engine
POOL_SUB_TILES = (1,)
N_GROUPS = 18                           # stats cols: u = 4t+g main, 16/17 strip
BN_COLS = 12 * len(BN_GROUPS)

F32 = mybir.dt.float32
F16 = mybir.dt.float16
SUB = mybir.AluOpType.subtract
SQUARE = mybir.ActivationFunctionType.Square


def _solve_cubic(Q: float) -> float:
    c = -Q
    for _ in range(5):
        f = c ** 3 / 6.0 + c + Q
        df = 0.5 * c * c + 1.0
        c = c - f / df
    return c


C1 = _solve_cubic((B_STR - 0.0) / ALPHA)
C2 = _solve_cubic((B_STR - 1.0) / ALPHA)


def _stencil_coeffs(S: np.ndarray):
    """Per-row stencil coefficients / C_T (c folded out; re-applied via host mask)."""
    S = S.astype(np.float64)
    dS = 1.0 / (N_S - 1)
    L = C2 * S + C1 * (1.0 - S)
    dL = C2 - C1
    S_u = ALPHA * dL * (0.5 * L ** 2 + 1.0)
    S_uu = ALPHA * dL ** 2 * L
    e = 0.5 * SIGMA ** 2 * S ** 2
    f = R * S
    a_uu = e / S_u ** 2
    a_u = f / S_u - e * S_uu / S_u ** 3
    hi = a_uu / dS ** 2 + a_u / (2 * dS)
    lo = a_uu / dS ** 2 - a_u / (2 * dS)
    mid = -2.0 * a_uu / dS ** 2 - R
    return lo / C_T, mid / C_T, hi / C_T


_PROGRAM = None


def _patch_tail(tc_cls):
    """Cheaper kernel tail: drain + single barrier, no per-sem HW clears.
    Semaphore bookkeeping (free/poison) is kept so scheduling stays valid."""
    from concourse.vector_clock import ScopedClock as _SC

    def _drain_and_barrier(self, tick_clock, wait_clock):
        drain_inst = self.nc.sync.drain()
        wait_clock.add_sem_waits(drain_inst.ins, _SC({None: tick_clock.global_clock}))
        self.nc.all_engine_barrier()
        popped = self.nc._tile_sem_poison_stack.pop()
        assert popped is self._sem_poison
        sems = list(self.sems.allocated().values())
        sem_nums = [s.num if hasattr(s, "num") else s for s in sems]
        self.nc._state.prepend_free_semaphores(sem_nums)
        for poison_set in self.nc._tile_sem_poison_stack:
            poison_set.update(sem_nums)

    tc_cls._drain_and_barrier = _drain_and_barrier


def _build_program():
    if os.environ.get("BSLOSS_FAST_TAIL", "1") == "1":
        _patch_tail(tile.TileContext)
    nc = bacc.Bacc("TRN2", target_bir_lowering=False)

    v_in = nc.dram_tensor("v_in", [IN_ROWS, N_T], F16, kind="ExternalInput")
    w_in = nc.dram_tensor("w_in", [P, W_COLS], F16, kind="ExternalInput")
    stats_out = nc.dram_tensor("stats_out", [P, N_GROUPS], F32, kind="ExternalOutput")
    bn_out = nc.dram_tensor("bn_out", [P, BN_COLS], F32, kind="ExternalOutput")

    with tile.TileContext(nc) as tc:
        with (
            tc.tile_pool(name="vpool", bufs=1) as vpool,
            tc.tile_pool(name="wpool", bufs=1) as wpool,
            tc.tile_pool(name="dpool", bufs=2) as dpool,
            tc.tile_pool(name="sqpool", bufs=1) as sqpool,
            tc.tile_pool(name="psum", bufs=4, space="PSUM") as psum_pool,
        ):
            wall = wpool.tile([P, W_COLS], F16)
            stats = wpool.tile([P, N_GROUPS], F32)
            bn = wpool.tile([P, BN_COLS], F32)

            # ---- DMAs, spread across the two HWDGE queues ----
            vs = vpool.tile([20, STRIP_W + 2], F16, tag="vs")
            vt = {t: vpool.tile([P, N_T], F16, tag=f"v{t}", name=f"v{t}")
                  for t in range(4)}

            # strip chunks on both queues in parallel, then w, then tiles
            nc.sync.dma_start(vs[0:10, 0:STRIP_W + 2],
                              v_in[STRIP_START:STRIP_START + 10, 0:STRIP_W + 2])
            nc.scalar.dma_start(vs[10:20, 0:STRIP_W + 2],
                                v_in[STRIP_START:STRIP_START + 10,
                                     STRIP_W:2 * STRIP_W + 2])
            nc.sync.dma_start(wall[:], w_in[:])

            def dma_tile(t, eng):
                r0 = TILE_STARTS[t]
                eng.dma_start(vt[t][:, 0:H0_W], v_in[r0:r0 + P, 0:H0_W])
                eng.dma_start(vt[t][:, H0_W:N_T], v_in[r0:r0 + P, H0_W:N_T])

            dma_tile(0, nc.scalar)      # ACT queue idle early; t0 needed first
            dma_tile(1, nc.sync)
            dma_tile(2, nc.sync)
            dma_tile(3, nc.scalar)

            ident = wall[0:P, W_IDENT:W_IDENT + P]

            def chunks(w):
                return [(512 * ci, min(512, w - 512 * ci))
                        for ci in range((w + 511) // 512)]

            def emit_tris(kdim, tri, rhs_v, groups):
                """One ring tile + tri matmuls per group; returns ps list.
                Batched so the PE runs long same-stationary streaks with
                few semaphore waits."""
                pss = []
                for f0, w in groups:
                    ps = psum_pool.tile([P, 1024], F32, tag="ps")
                    for off, cw in chunks(w):
                        nc.tensor.matmul(ps[0:kdim, off:off + cw], lhsT=tri,
                                         rhs=rhs_v[0:kdim, f0 + off:f0 + off + cw],
                                         start=True, stop=False)
                    pss.append(ps)
                return pss

            def emit_ident(kdim, ps, w, d, doff):
                for off, cw in chunks(w):
                    nc.tensor.matmul(ps[0:kdim, off:off + cw],
                                     lhsT=ident[0:kdim, 0:kdim],
                                     rhs=d[0:kdim, doff + off:doff + off + cw],
                                     start=False, stop=True)

            def emit_act(kdim, ps, w, u):
                sq = sqpool.tile([P, 1024], F32, tag="sq")
                nc.scalar.activation(sq[0:kdim, 0:w], ps[0:kdim, 0:w],
                                     SQUARE, accum_out=stats[0:kdim, u:u + 1])

            def emit_bn(ps, w, bi):
                for ci, (off, cw) in enumerate(chunks(w)):
                    nc.vector.bn_stats(bn[:, 12 * bi + 6 * ci:12 * bi + 6 * ci + 6],
                                       ps[:, off:off + cw])

            bn_idx = {tg: i for i, tg in enumerate(BN_GROUPS)}

            def emit_subs(t):
                d = dpool.tile([P, 4096], F16, tag="d", name=f"d{t}")
                eng = nc.gpsimd if t in POOL_SUB_TILES else nc.vector
                for p in (0, 1):
                    c0, w = PAIR_C0[p], PAIR_W[p]
                    eng.tensor_tensor(out=d[:, c0 - 1:c0 - 1 + w],
                                      in0=vt[t][:, c0 + 1:c0 + 1 + w],
                                      in1=vt[t][:, c0 - 1:c0 - 1 + w],
                                      op=SUB)
                return d

            # ---- strip: sub, two 1024-ish groups, ACT consumers ----
            ds = dpool.tile([20, STRIP_W + 1], F16, tag="ds")
            nc.vector.tensor_tensor(out=ds[0:20, 0:STRIP_W],
                                    in0=vs[0:20, 2:STRIP_W + 2],
                                    in1=vs[0:20, 0:STRIP_W], op=SUB)
            stri = wall[0:20, W_STRIP:W_STRIP + 20]
            strip_groups = ((1, 1024), (1025, STRIP_W - 1024))
            spss = emit_tris(20, stri, vs, strip_groups)
            for a, (f0, aw) in enumerate(strip_groups):
                emit_ident(20, spss[a], aw, ds, f0 - 1)
                emit_act(20, spss[a], aw, 16 + a)

            # ---- main tiles; bn consumers for tile t-1 are emitted after
            # tile t's subs so they never stall the DVE queue ahead of
            # fresh sub work.
            pending_bn = []
            main_groups = list(zip(GROUP_C0, GROUP_W))
            for t in range(4):
                tri = wall[0:P, 128 * t:128 * (t + 1)]
                d = emit_subs(t)
                for args in pending_bn:
                    emit_bn(*args)
                pending_bn = []
                pss = emit_tris(P, tri, vt[t], main_groups)
                for g in range(4):
                    c0, w = GROUP_C0[g], GROUP_W[g]
                    emit_ident(P, pss[g], w, d, c0 - 1)
                    u = 4 * t + g
                    if (t, g) in bn_idx:
                        pending_bn.append((pss[g], w, bn_idx[(t, g)]))
                    else:
                        emit_act(P, pss[g], w, u)
                if t == 2:
                    # early partial stats drain: tiles 0-1 + strip are final
                    nc.sync.dma_start(stats_out[:, 0:8], stats[:, 0:8])
                    nc.sync.dma_start(stats_out[:, 16:18], stats[:, 16:18])
            for args in pending_bn:
                emit_bn(*args)

            nc.sync.dma_start(stats_out[:, 8:16], stats[:, 8:16])
            nc.sync.dma_start(bn_out[:], bn[:])

    nc.compile()
    return nc


def _host_inputs_and_masks(V: np.ndarray, S: np.ndarray):
    lo, mid, hi = _stencil_coeffs(S)
    c2 = float(C_T) ** 2

    in_maps = []
    masks = []

    for c in range(N_CORES):
        rows = np.clip(np.arange(512 * c - 1, 512 * c + 513), 0, N_S - 1)
        v_shard = V[rows, :].astype(np.float16)

        w64 = np.zeros((P, W_COLS), np.float64)
        w64[:, W_IDENT:W_IDENT + P] = np.eye(P)
        mask = np.zeros((P, N_GROUPS), np.float32)
        for t in range(4):
            t0 = TILE_STARTS[t]
            for m in range(1, 127):
                g = 512 * c - 1 + t0 + m
                if not (1 <= g <= N_S - 2):
                    continue
                w64[m - 1, 128 * t + m] = lo[g]
                w64[m, 128 * t + m] = mid[g]
                w64[m + 1, 128 * t + m] = hi[g]
                mask[m, 4 * t:4 * t + 4] = c2
        for r in range(1, 9):
            g = 512 * c - 1 + STRIP_START + r
            if not (1 <= g <= N_S - 2):
                continue
            for j in range(2):
                w64[10 * j + r - 1, W_STRIP + 10 * j + r] = lo[g]
                w64[10 * j + r, W_STRIP + 10 * j + r] = mid[g]
                w64[10 * j + r + 1, W_STRIP + 10 * j + r] = hi[g]
                mask[10 * j + r, 16] = c2
                mask[10 * j + r, 17] = c2
        in_maps.append({"v_in": v_shard, "w_in": w64.astype(np.float16)})
        masks.append(mask)
    return in_maps, masks


_LAST_RESULTS = None  # stashed BassKernelResults (for the test harness)


def kernel(V_norm: np.ndarray, S_grid: np.ndarray, t_grid: np.ndarray):
    global _PROGRAM, _LAST_RESULTS

    V = np.asarray(V_norm, dtype=np.float32).reshape(N_S, N_T)
    S = np.asarray(S_grid, dtype=np.float32).reshape(N_S)
    t = np.asarray(t_grid, dtype=np.float32).reshape(N_T)

    if _PROGRAM is None:
        _PROGRAM = _build_program()
    nc = _PROGRAM

    in_maps, masks = _host_inputs_and_masks(V, S)
    trace = bool(os.environ.get("BSLOSS_TRACE"))
    res = run_bass_kernel_spmd(nc, in_maps, core_ids=list(range(N_CORES)),
                               trace=trace)
    _LAST_RESULTS = res

    pde_sum = 0.0
    for c in range(N_CORES):
        stats = res.results[c]["stats_out"].astype(np.float64)
        bn = res.results[c]["bn_out"].astype(np.float64)
        per_part = stats
        for bi, (bt, bg) in enumerate(BN_GROUPS):
            u = 4 * bt + bg
            rec = bn[:, 12 * bi:12 * bi + 12].reshape(P, 2, 2, 3)
            n_, mean_, m2_ = rec[..., 0], rec[..., 1], rec[..., 2]
            per_part[:, u] = (m2_ + n_ * mean_ * mean_).sum(axis=(1, 2))
        m = masks[c].astype(np.float64)
        pde_sum += float(np.where(m > 0, per_part * m, 0.0).sum())
    n_int = (N_S - 2) * (N_T - 2)
    pde_loss = pde_sum / n_int

    # ---- boundary losses on host (tiny O(N) edge terms), float64 ----
    V64 = V.astype(np.float64)
    S64 = S.astype(np.float64)
    t64 = t.astype(np.float64)

    loss_S0 = float((V64[0, :] ** 2).sum() / N_T)

    tau = 1.0 - t64
    V_ff = 1.0 - K * np.exp(-R * tau) / SMAX
    loss_Smax = float(((V64[N_S - 1, :] - V_ff) ** 2).sum() / N_T)

    x = SOFTPLUS_BETA * (S64 - K / SMAX)
    payoff = (np.maximum(x, 0.0) + np.log1p(np.exp(-np.abs(x)))) / SOFTPLUS_BETA
    diff_T = V64[:, N_T - 1] - payoff
    abs_d = np.abs(diff_T)
    huber = np.where(abs_d < HUBER_DELTA, 0.5 * diff_T ** 2,
                     HUBER_DELTA * (abs_d - 0.5 * HUBER_DELTA))
    loss_T = float(huber.sum() / N_S)

    total = L_PDE * pde_loss + L_BC * loss_Smax + L_TC * loss_T
    return (np.float32(total), np.float32(pde_loss), np.float32(loss_S0),
            np.float32(loss_Smax), np.float32(loss_T))


# revision 23
# speedup vs baseline: 1.0511x; 1.0511x over previous
"""Trainium2 Bass kernel for nn_BSLoss (Black-Scholes PINN loss on a 4096x4096 grid).

Strategy V3 (8 NeuronCores, SPMD, S-sharded, fp16 on device):
  - Host converts V to fp16 (halves DMA: ~4.4 MB/core). The loss is a mean of
    ~16.7M squared residuals, so V-rounding noise enters as E[eps^2]/E[r^2]
    ~ 2^-22 -- far below the 2e-2 gate.
  - Each core: 512 grid rows (+1-row halos) x 4096 t-cols as 4 x [128, 4096]
    tiles (output rows 1..126 each) + the last 10 rows folded as 2
    column-chunks x 10 rows = [20, 2049] (4094 = 2*2047 exactly, so the fold
    has no garbage columns).
  - Work unit = supergroup of 2048 output cols ([128, 2048] PSUM, 4 banks,
    ring of 2). Per supergroup: DVE computes D = V[:,t+1]-V[:,t-1] in one
    wide fp16 TT (2x mode); PE runs 4x512 tridiagonal fp16 matmuls
    (S-stencil) then 4 identity matmuls accumulating D into the same banks;
    the consumer is either one wide ScalarE activation(Square, accum_out) or
    4 DVE bn_stats (chosen per supergroup to balance ACT vs DVE).
  - DMA issue is spread across the SP, Activation, and GPSIMD DGE queues so
    descriptor generation doesn't serialize behind one queue.
  - Host applies per-row masks (x C_T^2, the folded-out time-step scale),
    reconstructs bn sums (M2 + n*mean^2), and computes the O(N) boundary
    losses in float64.
"""
import os
import sys

if "/opt/trn_rl_repo" not in sys.path:
    sys.path.insert(0, "/opt/trn_rl_repo")

import numpy as np

import concourse.mybir as mybir
import concourse.tile as tile
from concourse import bacc
from concourse.bass_utils import run_bass_kernel_spmd

# ---- problem constants (match the reference) ----
N_S, N_T = 4096, 4096
R, SIGMA, K, T_MAT, SMAX = 0.05, 0.2, 100.0, 1.0, 300.0
B_STR, ALPHA = K / SMAX, 0.5
L_PDE, L_BC, L_TC = 1.0, 10.0, 10.0
HUBER_DELTA = 0.01
SOFTPLUS_BETA = 50.0

N_CORES = 8
ROWS_PER_CORE = N_S // N_CORES          # 512
IN_ROWS = ROWS_PER_CORE + 2             # 514 (with halos)
P = 128
TILE_STARTS = [0, 126, 252, 378]        # full tiles; outputs local rows 1..504
STRIP_START = 504                       # strip rows 504..513 -> outputs 505..512
C_T = (N_T - 1) / 2.0 / T_MAT           # 2047.5

# weight layout (fp16): 4 tri blocks, identity, folded strip tri
W_IDENT = 512                           # cols 512..640: eye(128)
W_STRIP = 640                           # cols 640..660: [20,20] block-tridiag
W_COLS = 768

# main groups: output cols c0..c0+w-1, 1024-wide (1 PSUM bank pair, ring 4)
GROUP_C0 = [1, 1025, 2049, 3073]
GROUP_W = [1024, 1024, 1024, 1022]
# t-diff subs are emitted per pair (2048 wide); DMA halves overlap by 2 cols
# so the front pair's sub needs only half 0
PAIR_C0 = [1, 2049]
PAIR_W = [2048, 2046]
H0_W = 2050
# strip folded as 2 column-chunks x 10 rows: outputs f=1..2047 per chunk,
# chunk j <-> global cols 2047*j + f
STRIP_W = 2047
# groups whose sum-of-squares runs on DVE bn_stats instead of ACT (two in
# tile 3 so the tail isn't squeezed through ACT alone)
BN_GROUPS = [(0, 2), (1, 0), (1, 2), (2, 2), (3, 1), (3, 3)]
# tiles whose t-diff subs run on the (slow but otherwise idle) Pool engine.
# Measured: a 2046-col Pool TT takes 3.6us and stalls the PE's ident
# matmuls behind it -- keep empty.
POOL_SUB_TILES = ()
N_GROUPS = 18                           # stats cols: u = 4t+g main, 16/17 strip
BN_COLS = 12 * len(BN_GROUPS)

F32 = mybir.dt.float32
F16 = mybir.dt.float16
SUB = mybir.AluOpType.subtract
SQUARE = mybir.ActivationFunctionType.Square


def _solve_cubic(Q: float) -> float:
    c = -Q
    for _ in range(5):
        f = c ** 3 / 6.0 + c + Q
        df = 0.5 * c * c + 1.0
        c = c - f / df
    return c


C1 = _solve_cubic((B_STR - 0.0) / ALPHA)
C2 = _solve_cubic((B_STR - 1.0) / ALPHA)


def _stencil_coeffs(S: np.ndarray):
    """Per-row stencil coefficients / C_T (c folded out; re-applied via host mask)."""
    S = S.astype(np.float64)
    dS = 1.0 / (N_S - 1)
    L = C2 * S + C1 * (1.0 - S)
    dL = C2 - C1
    S_u = ALPHA * dL * (0.5 * L ** 2 + 1.0)
    S_uu = ALPHA * dL ** 2 * L
    e = 0.5 * SIGMA ** 2 * S ** 2
    f = R * S
    a_uu = e / S_u ** 2
    a_u = f / S_u - e * S_uu / S_u ** 3
    hi = a_uu / dS ** 2 + a_u / (2 * dS)
    lo = a_uu / dS ** 2 - a_u / (2 * dS)
    mid = -2.0 * a_uu / dS ** 2 - R
    return lo / C_T, mid / C_T, hi / C_T


_PROGRAM = None


def _patch_tail(tc_cls):
    """Cheaper kernel tail: drain + single barrier, no per-sem HW clears.
    Semaphore bookkeeping (free/poison) is kept so scheduling stays valid."""
    from concourse.vector_clock import ScopedClock as _SC

    def _drain_and_barrier(self, tick_clock, wait_clock):
        drain_inst = self.nc.sync.drain()
        wait_clock.add_sem_waits(drain_inst.ins, _SC({None: tick_clock.global_clock}))
        self.nc.all_engine_barrier()
        popped = self.nc._tile_sem_poison_stack.pop()
        assert popped is self._sem_poison
        sems = list(self.sems.allocated().values())
        sem_nums = [s.num if hasattr(s, "num") else s for s in sems]
        self.nc._state.prepend_free_semaphores(sem_nums)
        for poison_set in self.nc._tile_sem_poison_stack:
            poison_set.update(sem_nums)

    tc_cls._drain_and_barrier = _drain_and_barrier


def _build_program():
    if os.environ.get("BSLOSS_FAST_TAIL", "1") == "1":
        _patch_tail(tile.TileContext)
    nc = bacc.Bacc("TRN2", target_bir_lowering=False)

    v_in = nc.dram_tensor("v_in", [IN_ROWS, N_T], F16, kind="ExternalInput")
    w_in = nc.dram_tensor("w_in", [P, W_COLS], F16, kind="ExternalInput")
    stats_out = nc.dram_tensor("stats_out", [P, N_GROUPS], F32, kind="ExternalOutput")
    bn_out = nc.dram_tensor("bn_out", [P, BN_COLS], F32, kind="ExternalOutput")

    with tile.TileContext(nc) as tc:
        with (
            tc.tile_pool(name="vpool", bufs=1) as vpool,
            tc.tile_pool(name="wpool", bufs=1) as wpool,
            tc.tile_pool(name="dpool", bufs=2) as dpool,
            tc.tile_pool(name="sqpool", bufs=1) as sqpool,
            tc.tile_pool(name="psum", bufs=4, space="PSUM") as psum_pool,
        ):
            wall = wpool.tile([P, W_COLS], F16)
            stats = wpool.tile([P, N_GROUPS], F32)
            bn = wpool.tile([P, BN_COLS], F32)

            # ---- DMAs, spread across the two HWDGE queues ----
            vs = vpool.tile([20, STRIP_W + 2], F16, tag="vs")
            vt = {t: vpool.tile([P, N_T], F16, tag=f"v{t}", name=f"v{t}")
                  for t in range(4)}

            # strip chunks on both queues in parallel, then w, then tiles
            nc.sync.dma_start(vs[0:10, 0:STRIP_W + 2],
                              v_in[STRIP_START:STRIP_START + 10, 0:STRIP_W + 2])
            nc.scalar.dma_start(vs[10:20, 0:STRIP_W + 2],
                                v_in[STRIP_START:STRIP_START + 10,
                                     STRIP_W:2 * STRIP_W + 2])
            nc.sync.dma_start(wall[:], w_in[:])

            def dma_tile(t, eng):
                r0 = TILE_STARTS[t]
                eng.dma_start(vt[t][:, 0:H0_W], v_in[r0:r0 + P, 0:H0_W])
                eng.dma_start(vt[t][:, H0_W:N_T], v_in[r0:r0 + P, H0_W:N_T])

            dma_tile(0, nc.scalar)      # ACT queue idle early; t0 needed first
            dma_tile(1, nc.sync)
            dma_tile(2, nc.sync)
            dma_tile(3, nc.scalar)

            ident = wall[0:P, W_IDENT:W_IDENT + P]

            def chunks(w):
                return [(512 * ci, min(512, w - 512 * ci))
                        for ci in range((w + 511) // 512)]

            def emit_tris(kdim, tri, rhs_v, groups):
                """One ring tile + tri matmuls per group; returns ps list.
                Batched so the PE runs long same-stationary streaks with
                few semaphore waits."""
                pss = []
                for f0, w in groups:
                    ps = psum_pool.tile([P, 1024], F32, tag="ps")
                    for off, cw in chunks(w):
                        nc.tensor.matmul(ps[0:kdim, off:off + cw], lhsT=tri,
                                         rhs=rhs_v[0:kdim, f0 + off:f0 + off + cw],
                                         start=True, stop=False)
                    pss.append(ps)
                return pss

            def emit_ident(kdim, ps, w, d, doff):
                for off, cw in chunks(w):
                    nc.tensor.matmul(ps[0:kdim, off:off + cw],
                                     lhsT=ident[0:kdim, 0:kdim],
                                     rhs=d[0:kdim, doff + off:doff + off + cw],
                                     start=False, stop=True)

            def emit_act(kdim, ps, w, u):
                sq = sqpool.tile([P, 1024], F32, tag="sq")
                nc.scalar.activation(sq[0:kdim, 0:w], ps[0:kdim, 0:w],
                                     SQUARE, accum_out=stats[0:kdim, u:u + 1])

            def emit_bn(ps, w, bi):
                for ci, (off, cw) in enumerate(chunks(w)):
                    nc.vector.bn_stats(bn[:, 12 * bi + 6 * ci:12 * bi + 6 * ci + 6],
                                       ps[:, off:off + cw])

            bn_idx = {tg: i for i, tg in enumerate(BN_GROUPS)}

            def emit_subs(t):
                d = dpool.tile([P, 4096], F16, tag="d", name=f"d{t}")
                eng = nc.gpsimd if t in POOL_SUB_TILES else nc.vector
                for p in (0, 1):
                    c0, w = PAIR_C0[p], PAIR_W[p]
                    eng.tensor_tensor(out=d[:, c0 - 1:c0 - 1 + w],
                                      in0=vt[t][:, c0 + 1:c0 + 1 + w],
                                      in1=vt[t][:, c0 - 1:c0 - 1 + w],
                                      op=SUB)
                return d

            # ---- strip: sub, two 1024-ish groups, ACT consumers ----
            ds = dpool.tile([20, STRIP_W + 1], F16, tag="ds")
            nc.vector.tensor_tensor(out=ds[0:20, 0:STRIP_W],
                                    in0=vs[0:20, 2:STRIP_W + 2],
                                    in1=vs[0:20, 0:STRIP_W], op=SUB)
            stri = wall[0:20, W_STRIP:W_STRIP + 20]
            strip_groups = ((1, 1024), (1025, STRIP_W - 1024))
            spss = emit_tris(20, stri, vs, strip_groups)
            for a, (f0, aw) in enumerate(strip_groups):
                emit_ident(20, spss[a], aw, ds, f0 - 1)
                emit_act(20, spss[a], aw, 16 + a)

            # ---- main tiles; bn consumers for tile t-1 are emitted after
            # tile t's subs so they never stall the DVE queue ahead of
            # fresh sub work.
            pending_bn = []
            main_groups = list(zip(GROUP_C0, GROUP_W))
            for t in range(4):
                tri = wall[0:P, 128 * t:128 * (t + 1)]
                d = emit_subs(t)
                for args in pending_bn:
                    emit_bn(*args)
                pending_bn = []
                pss = emit_tris(P, tri, vt[t], main_groups)
                for g in range(4):
                    c0, w = GROUP_C0[g], GROUP_W[g]
                    emit_ident(P, pss[g], w, d, c0 - 1)
                    u = 4 * t + g
                    if (t, g) in bn_idx:
                        pending_bn.append((pss[g], w, bn_idx[(t, g)]))
                    else:
                        emit_act(P, pss[g], w, u)
                if t == 2:
                    # early partial stats drain: tiles 0-1 + strip are final
                    nc.sync.dma_start(stats_out[:, 0:8], stats[:, 0:8])
                    nc.sync.dma_start(stats_out[:, 16:18], stats[:, 16:18])
            for args in pending_bn:
                emit_bn(*args)

            nc.sync.dma_start(stats_out[:, 8:16], stats[:, 8:16])
            nc.sync.dma_start(bn_out[:], bn[:])

    nc.compile()
    return nc


def _host_inputs_and_masks(V: np.ndarray, S: np.ndarray):
    lo, mid, hi = _stencil_coeffs(S)
    c2 = float(C_T) ** 2

    in_maps = []
    masks = []

    for c in range(N_CORES):
        rows = np.clip(np.arange(512 * c - 1, 512 * c + 513), 0, N_S - 1)
        v_shard = V[rows, :].astype(np.float16)

        w64 = np.zeros((P, W_COLS), np.float64)
        w64[:, W_IDENT:W_IDENT + P] = np.eye(P)
        mask = np.zeros((P, N_GROUPS), np.float32)
        for t in range(4):
            t0 = TILE_STARTS[t]
            for m in range(1, 127):
                g = 512 * c - 1 + t0 + m
                if not (1 <= g <= N_S - 2):
                    continue
                w64[m - 1, 128 * t + m] = lo[g]
                w64[m, 128 * t + m] = mid[g]
                w64[m + 1, 128 * t + m] = hi[g]
                mask[m, 4 * t:4 * t + 4] = c2
        for r in range(1, 9):
            g = 512 * c - 1 + STRIP_START + r
            if not (1 <= g <= N_S - 2):
                continue
            for j in range(2):
                w64[10 * j + r - 1, W_STRIP + 10 * j + r] = lo[g]
                w64[10 * j + r, W_STRIP + 10 * j + r] = mid[g]
                w64[10 * j + r + 1, W_STRIP + 10 * j + r] = hi[g]
                mask[10 * j + r, 16] = c2
                mask[10 * j + r, 17] = c2
        in_maps.append({"v_in": v_shard, "w_in": w64.astype(np.float16)})
        masks.append(mask)
    return in_maps, masks


_LAST_RESULTS = None  # stashed BassKernelResults (for the test harness)


def kernel(V_norm: np.ndarray, S_grid: np.ndarray, t_grid: np.ndarray):
    global _PROGRAM, _LAST_RESULTS

    V = np.asarray(V_norm, dtype=np.float32).reshape(N_S, N_T)
    S = np.asarray(S_grid, dtype=np.float32).reshape(N_S)
    t = np.asarray(t_grid, dtype=np.float32).reshape(N_T)

    if _PROGRAM is None:
        _PROGRAM = _build_program()
    nc = _PROGRAM

    in_maps, masks = _host_inputs_and_masks(V, S)
    trace = bool(os.environ.get("BSLOSS_TRACE"))
    res = run_bass_kernel_spmd(nc, in_maps, core_ids=list(range(N_CORES)),
                               trace=trace)
    _LAST_RESULTS = res

    pde_sum = 0.0
    for c in range(N_CORES):
        stats = res.results[c]["stats_out"].astype(np.float64)
        bn = res.results[c]["bn_out"].astype(np.float64)
        per_part = stats
        for bi, (bt, bg) in enumerate(BN_GROUPS):
            u = 4 * bt + bg
            rec = bn[:, 12 * bi:12 * bi + 12].reshape(P, 2, 2, 3)
            n_, mean_, m2_ = rec[..., 0], rec[..., 1], rec[..., 2]
            per_part[:, u] = (m2_ + n_ * mean_ * mean_).sum(axis=(1, 2))
        m = masks[c].astype(np.float64)
        pde_sum += float(np.where(m > 0, per_part * m, 0.0).sum())
    n_int = (N_S - 2) * (N_T - 2)
    pde_loss = pde_sum / n_int

    # ---- boundary losses on host (tiny O(N) edge terms), float64 ----
    V64 = V.astype(np.float64)
    S64 = S.astype(np.float64)
    t64 = t.astype(np.float64)

    loss_S0 = float((V64[0, :] ** 2).sum() / N_T)

    tau = 1.0 - t64
    V_ff = 1.0 - K * np.exp(-R * tau) / SMAX
    loss_Smax = float(((V64[N_S - 1, :] - V_ff) ** 2).sum() / N_T)

    x = SOFTPLUS_BETA * (S64 - K / SMAX)
    payoff = (np.maximum(x, 0.0) + np.log1p(np.exp(-np.abs(x)))) / SOFTPLUS_BETA
    diff_T = V64[:, N_T - 1] - payoff
    abs_d = np.abs(diff_T)
    huber = np.where(abs_d < HUBER_DELTA, 0.5 * diff_T ** 2,
                     HUBER_DELTA * (abs_d - 0.5 * HUBER_DELTA))
    loss_T = float(huber.sum() / N_S)

    total = L_PDE * pde_loss + L_BC * loss_Smax + L_TC * loss_T
    return (np.float32(total), np.float32(pde_loss), np.float32(loss_S0),
            np.float32(loss_Smax), np.float32(loss_T))


# revision 24
# speedup vs baseline: 1.0678x; 1.0158x over previous
"""Trainium2 Bass kernel for nn_BSLoss (Black-Scholes PINN loss on a 4096x4096 grid).

Strategy V3 (8 NeuronCores, SPMD, S-sharded, fp16 on device):
  - Host converts V to fp16 (halves DMA: ~4.4 MB/core). The loss is a mean of
    ~16.7M squared residuals, so V-rounding noise enters as E[eps^2]/E[r^2]
    ~ 2^-22 -- far below the 2e-2 gate.
  - Each core: 512 grid rows (+1-row halos) x 4096 t-cols as 4 x [128, 4096]
    tiles (output rows 1..126 each) + the last 10 rows folded as 2
    column-chunks x 10 rows = [20, 2049] (4094 = 2*2047 exactly, so the fold
    has no garbage columns).
  - Work unit = supergroup of 2048 output cols ([128, 2048] PSUM, 4 banks,
    ring of 2). Per supergroup: DVE computes D = V[:,t+1]-V[:,t-1] in one
    wide fp16 TT (2x mode); PE runs 4x512 tridiagonal fp16 matmuls
    (S-stencil) then 4 identity matmuls accumulating D into the same banks;
    the consumer is either one wide ScalarE activation(Square, accum_out) or
    4 DVE bn_stats (chosen per supergroup to balance ACT vs DVE).
  - DMA issue is spread across the SP, Activation, and GPSIMD DGE queues so
    descriptor generation doesn't serialize behind one queue.
  - Host applies per-row masks (x C_T^2, the folded-out time-step scale),
    reconstructs bn sums (M2 + n*mean^2), and computes the O(N) boundary
    losses in float64.
"""
import os
import sys

if "/opt/trn_rl_repo" not in sys.path:
    sys.path.insert(0, "/opt/trn_rl_repo")

import numpy as np

import concourse.mybir as mybir
import concourse.tile as tile
from concourse import bacc
from concourse.bass_utils import run_bass_kernel_spmd

# ---- problem constants (match the reference) ----
N_S, N_T = 4096, 4096
R, SIGMA, K, T_MAT, SMAX = 0.05, 0.2, 100.0, 1.0, 300.0
B_STR, ALPHA = K / SMAX, 0.5
L_PDE, L_BC, L_TC = 1.0, 10.0, 10.0
HUBER_DELTA = 0.01
SOFTPLUS_BETA = 50.0

N_CORES = 8
ROWS_PER_CORE = N_S // N_CORES          # 512
IN_ROWS = ROWS_PER_CORE + 2             # 514 (with halos)
P = 128
TILE_STARTS = [0, 126, 252, 378]        # full tiles; outputs local rows 1..504
STRIP_START = 504                       # strip rows 504..513 -> outputs 505..512
C_T = (N_T - 1) / 2.0 / T_MAT           # 2047.5

# weight layout (fp16): 4 tri blocks, identity, folded strip tri
W_IDENT = 512                           # cols 512..640: eye(128)
W_STRIP = 640                           # cols 640..660: [20,20] block-tridiag
W_COLS = 768

# main groups: output cols c0..c0+w-1, 1024-wide (1 PSUM bank pair, ring 4)
GROUP_C0 = [1, 1025, 2049, 3073]
GROUP_W = [1024, 1024, 1024, 1022]
# t-diff subs are emitted per pair (2048 wide); DMA halves overlap by 2 cols
# so the front pair's sub needs only half 0
PAIR_C0 = [1, 2049]
PAIR_W = [2048, 2046]
H0_W = 2050
# strip folded as 2 column-chunks x 10 rows: outputs f=1..2047 per chunk,
# chunk j <-> global cols 2047*j + f
STRIP_W = 2047
# groups whose sum-of-squares runs on DVE bn_stats instead of ACT (two in
# tile 3 so the tail isn't squeezed through ACT alone)
BN_GROUPS = [(0, 2), (1, 2), (2, 2), (3, 1), (3, 3)]
N_GROUPS = 18                           # stats cols: u = 4t+g main, 16/17 strip
BN_COLS = 12 * len(BN_GROUPS)

F32 = mybir.dt.float32
F16 = mybir.dt.float16
SUB = mybir.AluOpType.subtract
SQUARE = mybir.ActivationFunctionType.Square


def _solve_cubic(Q: float) -> float:
    c = -Q
    for _ in range(5):
        f = c ** 3 / 6.0 + c + Q
        df = 0.5 * c * c + 1.0
        c = c - f / df
    return c


C1 = _solve_cubic((B_STR - 0.0) / ALPHA)
C2 = _solve_cubic((B_STR - 1.0) / ALPHA)


def _stencil_coeffs(S: np.ndarray):
    """Per-row stencil coefficients / C_T (c folded out; re-applied via host mask)."""
    S = S.astype(np.float64)
    dS = 1.0 / (N_S - 1)
    L = C2 * S + C1 * (1.0 - S)
    dL = C2 - C1
    S_u = ALPHA * dL * (0.5 * L ** 2 + 1.0)
    S_uu = ALPHA * dL ** 2 * L
    e = 0.5 * SIGMA ** 2 * S ** 2
    f = R * S
    a_uu = e / S_u ** 2
    a_u = f / S_u - e * S_uu / S_u ** 3
    hi = a_uu / dS ** 2 + a_u / (2 * dS)
    lo = a_uu / dS ** 2 - a_u / (2 * dS)
    mid = -2.0 * a_uu / dS ** 2 - R
    return lo / C_T, mid / C_T, hi / C_T


_PROGRAM = None


def _patch_tail(tc_cls):
    """Cheaper kernel tail: drain + single barrier, no per-sem HW clears.
    Semaphore bookkeeping (free/poison) is kept so scheduling stays valid."""
    from concourse.vector_clock import ScopedClock as _SC

    def _drain_and_barrier(self, tick_clock, wait_clock):
        drain_inst = self.nc.sync.drain()
        wait_clock.add_sem_waits(drain_inst.ins, _SC({None: tick_clock.global_clock}))
        self.nc.all_engine_barrier()
        popped = self.nc._tile_sem_poison_stack.pop()
        assert popped is self._sem_poison
        sems = list(self.sems.allocated().values())
        sem_nums = [s.num if hasattr(s, "num") else s for s in sems]
        self.nc._state.prepend_free_semaphores(sem_nums)
        for poison_set in self.nc._tile_sem_poison_stack:
            poison_set.update(sem_nums)

    tc_cls._drain_and_barrier = _drain_and_barrier


def _build_program():
    if os.environ.get("BSLOSS_FAST_TAIL", "1") == "1":
        _patch_tail(tile.TileContext)
    nc = bacc.Bacc("TRN2", target_bir_lowering=False)

    v_in = nc.dram_tensor("v_in", [IN_ROWS, N_T], F16, kind="ExternalInput")
    w_in = nc.dram_tensor("w_in", [P, W_COLS], F16, kind="ExternalInput")
    stats_out = nc.dram_tensor("stats_out", [P, N_GROUPS], F32, kind="ExternalOutput")
    bn_out = nc.dram_tensor("bn_out", [P, BN_COLS], F32, kind="ExternalOutput")

    with tile.TileContext(nc) as tc:
        with (
            tc.tile_pool(name="vpool", bufs=1) as vpool,
            tc.tile_pool(name="wpool", bufs=1) as wpool,
            tc.tile_pool(name="dpool", bufs=2) as dpool,
            tc.tile_pool(name="sqpool", bufs=1) as sqpool,
            tc.tile_pool(name="psum", bufs=4, space="PSUM") as psum_pool,
        ):
            wall = wpool.tile([P, W_COLS], F16)
            stats = wpool.tile([P, N_GROUPS], F32)
            bn = wpool.tile([P, BN_COLS], F32)

            # ---- DMAs, spread across the two HWDGE queues ----
            vs = vpool.tile([20, STRIP_W + 2], F16, tag="vs")
            vt = {t: vpool.tile([P, N_T], F16, tag=f"v{t}", name=f"v{t}")
                  for t in range(4)}

            # strip chunks on both queues in parallel, then w, then tiles
            nc.sync.dma_start(vs[0:10, 0:STRIP_W + 2],
                              v_in[STRIP_START:STRIP_START + 10, 0:STRIP_W + 2])
            nc.scalar.dma_start(vs[10:20, 0:STRIP_W + 2],
                                v_in[STRIP_START:STRIP_START + 10,
                                     STRIP_W:2 * STRIP_W + 2])
            nc.sync.dma_start(wall[:], w_in[:])

            def dma_tile(t, eng):
                r0 = TILE_STARTS[t]
                eng.dma_start(vt[t][:, 0:H0_W], v_in[r0:r0 + P, 0:H0_W])
                eng.dma_start(vt[t][:, H0_W:N_T], v_in[r0:r0 + P, H0_W:N_T])

            dma_tile(0, nc.scalar)      # ACT queue idle early; t0 needed first
            dma_tile(1, nc.sync)
            dma_tile(2, nc.sync)
            dma_tile(3, nc.scalar)

            ident = wall[0:P, W_IDENT:W_IDENT + P]

            def chunks(w):
                return [(512 * ci, min(512, w - 512 * ci))
                        for ci in range((w + 511) // 512)]

            def emit_tris(kdim, tri, rhs_v, groups):
                """One ring tile + tri matmuls per group; returns ps list.
                Batched so the PE runs long same-stationary streaks with
                few semaphore waits."""
                pss = []
                for f0, w in groups:
                    ps = psum_pool.tile([P, 1024], F32, tag="ps")
                    for off, cw in chunks(w):
                        nc.tensor.matmul(ps[0:kdim, off:off + cw], lhsT=tri,
                                         rhs=rhs_v[0:kdim, f0 + off:f0 + off + cw],
                                         start=True, stop=False)
                    pss.append(ps)
                return pss

            def emit_ident(kdim, ps, w, d, doff):
                for off, cw in chunks(w):
                    nc.tensor.matmul(ps[0:kdim, off:off + cw],
                                     lhsT=ident[0:kdim, 0:kdim],
                                     rhs=d[0:kdim, doff + off:doff + off + cw],
                                     start=False, stop=True)

            def emit_act(kdim, ps, w, u):
                sq = sqpool.tile([P, 1024], F32, tag="sq")
                nc.scalar.activation(sq[0:kdim, 0:w], ps[0:kdim, 0:w],
                                     SQUARE, accum_out=stats[0:kdim, u:u + 1])

            def emit_bn(ps, w, bi):
                for ci, (off, cw) in enumerate(chunks(w)):
                    nc.vector.bn_stats(bn[:, 12 * bi + 6 * ci:12 * bi + 6 * ci + 6],
                                       ps[:, off:off + cw])

            bn_idx = {tg: i for i, tg in enumerate(BN_GROUPS)}

            def emit_subs(t):
                d = dpool.tile([P, 4096], F16, tag="d", name=f"d{t}")
                for p in (0, 1):
                    c0, w = PAIR_C0[p], PAIR_W[p]
                    nc.vector.tensor_tensor(out=d[:, c0 - 1:c0 - 1 + w],
                                            in0=vt[t][:, c0 + 1:c0 + 1 + w],
                                            in1=vt[t][:, c0 - 1:c0 - 1 + w],
                                            op=SUB)
                return d

            # ---- strip: sub, two 1024-ish groups, ACT consumers ----
            ds = dpool.tile([20, STRIP_W + 1], F16, tag="ds")
            nc.vector.tensor_tensor(out=ds[0:20, 0:STRIP_W],
                                    in0=vs[0:20, 2:STRIP_W + 2],
                                    in1=vs[0:20, 0:STRIP_W], op=SUB)
            stri = wall[0:20, W_STRIP:W_STRIP + 20]
            strip_groups = ((1, 1024), (1025, STRIP_W - 1024))
            spss = emit_tris(20, stri, vs, strip_groups)
            for a, (f0, aw) in enumerate(strip_groups):
                emit_ident(20, spss[a], aw, ds, f0 - 1)
                emit_act(20, spss[a], aw, 16 + a)

            # ---- main tiles; bn consumers for tile t-1 are emitted after
            # tile t's subs so they never stall the DVE queue ahead of
            # fresh sub work.
            pending_bn = []
            main_groups = list(zip(GROUP_C0, GROUP_W))
            for t in range(4):
                tri = wall[0:P, 128 * t:128 * (t + 1)]
                d = emit_subs(t)
                for args in pending_bn:
                    emit_bn(*args)
                pending_bn = []
                pss = emit_tris(P, tri, vt[t], main_groups)
                for g in range(4):
                    c0, w = GROUP_C0[g], GROUP_W[g]
                    emit_ident(P, pss[g], w, d, c0 - 1)
                    u = 4 * t + g
                    if (t, g) in bn_idx:
                        pending_bn.append((pss[g], w, bn_idx[(t, g)]))
                    else:
                        emit_act(P, pss[g], w, u)
            for args in pending_bn:
                emit_bn(*args)

            nc.sync.dma_start(stats_out[:], stats[:])
            nc.sync.dma_start(bn_out[:], bn[:])

    nc.compile()
    return nc


def _host_inputs_and_masks(V: np.ndarray, S: np.ndarray):
    lo, mid, hi = _stencil_coeffs(S)
    c2 = float(C_T) ** 2

    in_maps = []
    masks = []

    for c in range(N_CORES):
        rows = np.clip(np.arange(512 * c - 1, 512 * c + 513), 0, N_S - 1)
        v_shard = V[rows, :].astype(np.float16)

        w64 = np.zeros((P, W_COLS), np.float64)
        w64[:, W_IDENT:W_IDENT + P] = np.eye(P)
        mask = np.zeros((P, N_GROUPS), np.float32)
        for t in range(4):
            t0 = TILE_STARTS[t]
            for m in range(1, 127):
                g = 512 * c - 1 + t0 + m
                if not (1 <= g <= N_S - 2):
                    continue
                w64[m - 1, 128 * t + m] = lo[g]
                w64[m, 128 * t + m] = mid[g]
                w64[m + 1, 128 * t + m] = hi[g]
                mask[m, 4 * t:4 * t + 4] = c2
        for r in range(1, 9):
            g = 512 * c - 1 + STRIP_START + r
            if not (1 <= g <= N_S - 2):
                continue
            for j in range(2):
                w64[10 * j + r - 1, W_STRIP + 10 * j + r] = lo[g]
                w64[10 * j + r, W_STRIP + 10 * j + r] = mid[g]
                w64[10 * j + r + 1, W_STRIP + 10 * j + r] = hi[g]
                mask[10 * j + r, 16] = c2
                mask[10 * j + r, 17] = c2
        in_maps.append({"v_in": v_shard, "w_in": w64.astype(np.float16)})
        masks.append(mask)
    return in_maps, masks


_LAST_RESULTS = None  # stashed BassKernelResults (for the test harness)


def kernel(V_norm: np.ndarray, S_grid: np.ndarray, t_grid: np.ndarray):
    global _PROGRAM, _LAST_RESULTS

    V = np.asarray(V_norm, dtype=np.float32).reshape(N_S, N_T)
    S = np.asarray(S_grid, dtype=np.float32).reshape(N_S)
    t = np.asarray(t_grid, dtype=np.float32).reshape(N_T)

    if _PROGRAM is None:
        _PROGRAM = _build_program()
    nc = _PROGRAM

    in_maps, masks = _host_inputs_and_masks(V, S)
    trace = bool(os.environ.get("BSLOSS_TRACE"))
    res = run_bass_kernel_spmd(nc, in_maps, core_ids=list(range(N_CORES)),
                               trace=trace)
    _LAST_RESULTS = res

    pde_sum = 0.0
    for c in range(N_CORES):
        stats = res.results[c]["stats_out"].astype(np.float64)
        bn = res.results[c]["bn_out"].astype(np.float64)
        per_part = stats
        for bi, (bt, bg) in enumerate(BN_GROUPS):
            u = 4 * bt + bg
            rec = bn[:, 12 * bi:12 * bi + 12].reshape(P, 2, 2, 3)
            n_, mean_, m2_ = rec[..., 0], rec[..., 1], rec[..., 2]
            per_part[:, u] = (m2_ + n_ * mean_ * mean_).sum(axis=(1, 2))
        m = masks[c].astype(np.float64)
        pde_sum += float(np.where(m > 0, per_part * m, 0.0).sum())
    n_int = (N_S - 2) * (N_T - 2)
    pde_loss = pde_sum / n_int

    # ---- boundary losses on host (tiny O(N) edge terms), float64 ----
    V64 = V.astype(np.float64)
    S64 = S.astype(np.float64)
    t64 = t.astype(np.float64)

    loss_S0 = float((V64[0, :] ** 2).sum() / N_T)

    tau = 1.0 - t64
    V_ff = 1.0 - K * np.exp(-R * tau) / SMAX
    loss_Smax = float(((V64[N_S - 1, :] - V_ff) ** 2).sum() / N_T)

    x = SOFTPLUS_BETA * (S64 - K / SMAX)
    payoff = (np.maximum(x, 0.0) + np.log1p(np.exp(-np.abs(x)))) / SOFTPLUS_BETA
    diff_T = V64[:, N_T - 1] - payoff
    abs_d = np.abs(diff_T)
    huber = np.where(abs_d < HUBER_DELTA, 0.5 * diff_T ** 2,
                     HUBER_DELTA * (abs_d - 0.5 * HUBER_DELTA))
    loss_T = float(huber.sum() / N_S)

    total = L_PDE * pde_loss + L_BC * loss_Smax + L_TC * loss_T
    return (np.float32(total), np.float32(pde_loss), np.float32(loss_S0),
            np.float32(loss_Smax), np.float32(loss_T))
